# revision 1
# baseline (speedup 1.0000x reference)
"""Trainium2 Bass kernel for nn_Attention (B=4, N=2048, C=768, H=12).

Sharding: 8 cores = 4 batches x 2 head-groups (6 heads each).
Each core computes, for its (batch b, head-group g):
    qT/kT = (W{q,k}_g @ x_b^T)          [384, 2048]  (scale folded into Wq)
    v     = x_b @ Wv_g^T                [2048, 384]  (v_bias folded into proj bias
                                                      since softmax rows sum to 1)
    per head h, q-block: scores computed transposed [k, q]
        p = exp(s)  (no max-subtraction: scores ~ N(0,1))
        out^T = [v_h | 1]^T @ p   -> row 64 = softmax sums
        out_n^T = out^T[0:64] / sums
    y_partial = out_n @ Wp_g^T + pb_eff     [2048, 768]
Host sums the two partials per batch (tensor-parallel unshard).

Schedule: all input DMAs go through HWDGE queues in dependency-critical
order (xT in 4 q-chunks; f=0 slices of host-prearranged f-major wq/wk
first) so PE compute starts ~2us in behind a clock-ramp warmup; attention
blocks run hp=0 across all q-chunks first, then hp=1/2 interleaved per
q-chunk, with the remaining q/k feature blocks, v tiles and output
projections drained as PE fill work inside the attention phase. exp is
split ACT/DVE (Schraudolph bf16 trick on DVE for 5/16 k-tiles) so neither
elementwise engine paces the PE. The last block normalizes straight from
PSUM in 128-col slices, emitting each projection tile as soon as its slice
is ready, with the projection bias folded in as a rank-1 matmul.
"""

import numpy as np
import ml_dtypes

import concourse.bass as bass
import concourse.tile as tile
from concourse import bacc, mybir
from concourse.bass import ds, ts
from concourse.bass_utils import run_bass_kernel_spmd

N_CORES = 8
B, N, C = 4, 2048, 768
H, HD = 12, 64
HPC, GF = 6, 384          # heads per core, features per group
SCALE = HD ** -0.5        # 1/8, exact power of two
BF16, F32 = mybir.dt.bfloat16, mybir.dt.float32
CP = C // 128             # 6 contraction partition-tiles
FP = GF // 128            # 3 feature partition-tiles per group
QB = 512                  # q block
NB = N // QB              # 4
NT = N // 128             # 16 token tiles
KT = N // 128             # 16 k tiles
PAIR_LAG = 3
EXPF = mybir.ActivationFunctionType.Exp
# NB: custom-DVE ops (reciprocal_approx_*, etc.) require partition-0-aligned
# input APs on hardware; stage via a plain tensor_copy first.
# k-tiles whose exp runs on DVE (bf16 Schraudolph: bits16 = A*x + B as bf16)
DVE_KT = frozenset({2, 5, 8, 11, 13})
SCH_A = 128.0 / np.log(2.0)
SCH_B = 16248.6
I16 = mybir.dt.int16
AVG = 4                   # attn@v burst size between score emissions


def _body(nc, tc, pools, aps):
    const, qkvp, pp, normp, yp, psA, psS, psO, dramp = pools
    xT, wqT, wkT, wvT, wpT, qb, pb, out = aps

    # ---- input DMAs: xT in 4 q-chunks so compute starts early; weights
    # interleaved on separate queues so wq/wk land before their first use.
    xT_sb = const.tile([128, CP, N], BF16, tag="xT")
    xT_r = xT.ap().rearrange("(t p) n -> p t n", p=128)
    # wq/wk arrive host-prearranged as [128, FP, CP, 128] (f-major) so the
    # f=0 slice is one DMA with >=1536B contiguous runs (full DMA bandwidth)
    wq_sb = const.tile([128, FP, CP, 128], BF16, tag="wq")
    qb_sb = const.tile([128, FP], F32, tag="qb")
    wk_sb = const.tile([128, FP, CP, 128], BF16, tag="wk")
    wv_sb = const.tile([128, CP, GF], BF16, tag="wv")
    wp_sb = const.tile([128, FP, C], BF16, tag="wp")
    pb_sb = const.tile([128, C], F32, tag="pb")

    # All input loads go through HWDGE queues (sync/scalar) — the gpsimd
    # queue is SWDGE (slow path + Pool-engine cost), keep it for small
    # mid-run staging DMAs only. Order = dependency-critical order; the
    # DMA engines drain roughly in trigger order across queues.
    nc.sync.dma_start(out=wq_sb[:, 0], in_=wqT.ap()[:, 0])
    nc.scalar.dma_start(out=qb_sb[:], in_=qb.ap().rearrange("(t p) -> p t", p=128))
    nc.sync.dma_start(out=xT_sb[:, :, ds(0, QB)], in_=xT_r[:, :, ds(0, QB)])
    nc.scalar.dma_start(out=wk_sb[:, 0], in_=wkT.ap()[:, 0])
    nc.scalar.dma_start(out=wv_sb[:], in_=wvT.ap().rearrange("(t p) n -> p t n", p=128))
    nc.sync.dma_start(out=xT_sb[:, :, ds(QB, QB)], in_=xT_r[:, :, ds(QB, QB)])
    nc.sync.dma_start(out=xT_sb[:, :, ds(2 * QB, QB)], in_=xT_r[:, :, ds(2 * QB, QB)])
    nc.sync.dma_start(out=xT_sb[:, :, ds(3 * QB, QB)], in_=xT_r[:, :, ds(3 * QB, QB)])
    nc.scalar.dma_start(out=wq_sb[:, 1:FP], in_=wqT.ap()[:, 1:FP])
    nc.scalar.dma_start(out=wk_sb[:, 1:FP], in_=wkT.ap()[:, 1:FP])
    nc.scalar.dma_start(out=wp_sb[:], in_=wpT.ap().rearrange("(t p) n -> p t n", p=128))
    pb_ap = pb.ap()
    pb_bcast = bass.AP(tensor=pb_ap.tensor, offset=pb_ap.offset, ap=[[0, 128]] + list(pb_ap.ap))
    nc.scalar.dma_start(out=pb_sb[:], in_=pb_bcast)

    qT_sb = qkvp.tile([128, FP, N], BF16, tag="qT")
    kT_sb = qkvp.tile([128, FP, N], BF16, tag="kT")
    v_sb = qkvp.tile([128, NT, HPC, HD + 1], BF16, tag="v")
    outT_sb = qkvp.tile([128, FP, N], BF16, tag="outT")

    # ones column for softmax sums
    nc.vector.memset(v_sb[:, :, :, HD], 1.0)
    ones64 = const.tile([1, 64], BF16, tag="ones64")
    nc.vector.memset(ones64[:], 1.0)
    ones128 = const.tile([1, 128], BF16, tag="ones128")
    nc.vector.memset(ones128[:], 1.0)
    pb_bf = const.tile([1, C], BF16, tag="pbbf")
    nc.vector.tensor_copy(pb_bf[:], pb_sb[0:1, :])

    # warm up the PE clock ramp while input DMAs stream: matmuls on a
    # zeroed slice of outT (not written until much later), result never read
    warm_sb = outT_sb[:, 0, 0:512]
    nc.vector.memset(warm_sb, 0.0)
    ps_w = psA.tile([128, 512], F32, tag="mm", name="warmup")
    for i in range(10):
        nc.tensor.matmul(
            ps_w[:], lhsT=outT_sb[:, 0, 0:128], rhs=warm_sb,
            start=(i == 0), stop=(i == 9),
        )

    # ---- qkv projection emitters (one PSUM group each)
    def qk_group(w_sb, f, nb, dst_sb, is_q):
        ps = psA.tile([128, QB], F32, tag="mm", name=f"qk{f}_{nb}_{int(is_q)}")
        for cp in range(CP):
            nc.tensor.matmul(
                ps[:],
                lhsT=w_sb[:, f, cp, :],
                rhs=xT_sb[:, cp, ds(nb * QB, QB)],
                start=(cp == 0),
                stop=(cp == CP - 1),
            )
        if is_q:
            nc.scalar.activation(
                dst_sb[:, f, ds(nb * QB, QB)], ps[:],
                mybir.ActivationFunctionType.Identity, bias=qb_sb[:, f : f + 1],
            )
        else:
            nc.scalar.copy(dst_sb[:, f, ds(nb * QB, QB)], ps[:])

    def v_group(nt):
        ps = psA.tile([128, GF], F32, tag="mm", name=f"v{nt}")
        for cp in range(CP):
            nc.tensor.matmul(
                ps[:],
                lhsT=xT_sb[:, cp, ts(nt, 128)],
                rhs=wv_sb[:, cp, :],
                start=(cp == 0),
                stop=(cp == CP - 1),
            )
        nc.scalar.copy(v_sb[:, nt, :, 0:HD], ps[:].rearrange("p (h d) -> p h d", h=HPC))

    # ---- attention machinery
    class Block:
        def __init__(self, nb, hp):
            self.nb, self.hp = nb, hp
            self.p_t = pp.tile([128, KT, 2, QB], BF16, tag="p", name=f"p{nb}_{hp}")
            self.pso = [
                psO.tile([128, QB], F32, tag="acc", name=f"pso{nb}_{hp}_{i}")
                for i in range(2)
            ]

    def emit_scores(blk, kt):
        ps_s = psS.tile([128, 2, QB], F32, tag="s", name=f"s{blk.nb}_{blk.hp}_{kt}")
        for hi in range(2):
            po = hi * 64
            nc.tensor.matmul(
                ps_s[:, hi, :],
                lhsT=kT_sb[po : po + 64, blk.hp, ts(kt, 128)],
                rhs=qT_sb[po : po + 64, blk.hp, ds(blk.nb * QB, QB)],
                start=True,
                stop=True,
                skip_group_check=True,
            )
        if kt in DVE_KT:
            nc.vector.tensor_scalar(
                out=blk.p_t[:, kt, :, :].bitcast(I16), in0=ps_s[:],
                scalar1=SCH_A, scalar2=SCH_B,
                op0=mybir.AluOpType.mult, op1=mybir.AluOpType.add,
            )
        else:
            nc.scalar.activation(blk.p_t[:, kt, :, :], ps_s[:], EXPF)

    def emit_av(blk, kt):
        for hi in range(2):
            h = 2 * blk.hp + hi
            nc.tensor.matmul(
                blk.pso[hi][0 : HD + 1, :],
                lhsT=v_sb[:, kt, h, :],
                rhs=blk.p_t[:, kt, hi, :],
                start=(kt == 0),
                stop=(kt == KT - 1),
                skip_group_check=True,
            )

    LAST_BLOCKS = {(NB - 1, FP - 1)}

    def emit_sums_copy(blk):
        # evacuate the [65, QB] accumulator to SBUF (frees the PSUM slot fast),
        # recip the sums row, stage it through DRAM for a partition-broadcast.
        blk.outU, blk.rec_dram = [], []
        for hi in range(2):
            outU = normp.tile([HD + 1, QB], BF16, tag="outU", name=f"oU{blk.nb}_{blk.hp}_{hi}", bufs=4)
            nc.vector.tensor_copy(outU[:], blk.pso[hi][0 : HD + 1, :])
            sr = normp.tile([1, QB], F32, tag="sumsrow", name=f"sr2{blk.nb}_{blk.hp}_{hi}", bufs=6)
            nc.vector.tensor_copy(sr[:], outU[HD : HD + 1, :])
            rr = normp.tile([1, QB], F32, tag="recrow", name=f"rr{blk.nb}_{blk.hp}_{hi}", bufs=6)
            nc.vector.reciprocal_approx_fast(rr[:], sr[:])
            if (blk.nb, blk.hp) in LAST_BLOCKS:
                rrb = normp.tile([1, QB], BF16, tag="recrowb", name=f"rb{blk.nb}_{blk.hp}_{hi}")
                nc.vector.tensor_copy(rrb[:], rr[:])
                blk.rec_dram.append(rrb)
            else:
                rd = dramp.tile([1, QB], F32, name=f"rd{blk.nb}_{blk.hp}_{hi}")
                nc.sync.dma_start(out=rd[:], in_=rr[:])
                blk.rec_dram.append(rd)
            blk.outU.append(outU)

    def emit_norm(blk):
        nb, hp = blk.nb, blk.hp
        for hi in range(2):
            rec_b = normp.tile([64, QB], F32, tag="rec", name=f"rec{nb}_{hp}_{hi}", bufs=3)
            if (nb, hp) in LAST_BLOCKS:
                rec_ps = psS.tile([64, QB], F32, tag="s", name=f"rps{nb}_{hp}_{hi}")
                nc.tensor.matmul(
                    rec_ps[:], lhsT=ones64[:], rhs=blk.rec_dram[hi][:],
                    start=True, stop=True, skip_group_check=True,
                )
                nc.vector.tensor_copy(rec_b[:], rec_ps[:])
            else:
                rd_ap = blk.rec_dram[hi][0:1, :]
                rd_b = bass.AP(
                    tensor=rd_ap.tensor, offset=rd_ap.offset,
                    ap=[[0, 64]] + list(rd_ap.ap)[1:],
                )
                nc.sync.dma_start(out=rec_b[:], in_=rd_b)
            if hi == 0:
                nc.vector.tensor_mul(
                    outT_sb[0:64, hp, ds(nb * QB, QB)], blk.outU[hi][0:HD, :], rec_b[:]
                )
            else:
                tmp = normp.tile([64, QB], BF16, tag="tmp", name=f"tmp{nb}_{hp}", bufs=4)
                nc.vector.tensor_mul(tmp[:], blk.outU[hi][0:HD, :], rec_b[:])
                eng = nc.sync if nb == NB - 1 else nc.gpsimd
                eng.dma_start(
                    out=outT_sb[64:128, hp, ds(nb * QB, QB)], in_=tmp[:]
                )

    def emit_final_norm_proj(blk):
        # last attention block: normalize straight from PSUM (no evac needed
        # at the tail) in 128-col slices, emitting each projection tile as
        # soon as its slice of outT is written — keeps PE fed through the
        # drain instead of idling behind the full-width norm chain.
        nb, hp = blk.nb, blk.hp
        rec_bs = {}
        for half in range(2):
            hq = ds(half * (QB // 2), QB // 2)
            for hi in range(2):
                sr = normp.tile([1, QB // 2], F32, tag="sumsrow", name=f"fsr{hi}_{half}", bufs=6)
                # hi=1 chain staged via ACT so the two chains overlap
                (nc.vector.tensor_copy if hi == 0 else nc.scalar.copy)(
                    sr[:], blk.pso[hi][HD : HD + 1, hq]
                )
                rr = normp.tile([1, QB // 2], F32, tag="recrow", name=f"frr{hi}_{half}", bufs=6)
                nc.vector.reciprocal_approx_fast(rr[:], sr[:])
                rrb = normp.tile([1, QB // 2], BF16, tag="recrowb", name=f"frb{hi}_{half}", bufs=2)
                (nc.vector.tensor_copy if hi == 0 else nc.scalar.copy)(rrb[:], rr[:])
                rec_ps = psS.tile([64, QB // 2], F32, tag="s", name=f"frp{hi}_{half}")
                nc.tensor.matmul(
                    rec_ps[:], lhsT=ones64[:], rhs=rrb[:],
                    start=True, stop=True, skip_group_check=True,
                )
                rec_b = normp.tile([64, QB // 2], F32, tag="rec", name=f"frec{hi}_{half}", bufs=3)
                (nc.vector.tensor_copy if hi == 0 else nc.scalar.copy)(
                    rec_b[:], rec_ps[:]
                )
                rec_bs[hi, half] = rec_b
            for s2 in range(2):
                s = 2 * half + s2
                sl = ds(nb * QB + s * 128, 128)
                nc.vector.tensor_mul(
                    outT_sb[0:64, hp, sl], blk.pso[0][0:HD, ts(s, 128)],
                    rec_bs[0, half][:, ts(s2, 128)],
                )
                tmp = normp.tile([64, 128], BF16, tag="tmp", name=f"ftmp{s}", bufs=4)
                nc.vector.tensor_mul(
                    tmp[:], blk.pso[1][0:HD, ts(s, 128)], rec_bs[1, half][:, ts(s2, 128)]
                )
                nc.sync.dma_start(out=outT_sb[64:128, hp, sl], in_=tmp[:])
                emit_proj_qt(4 * nb + s, tail=True)

    def emit_proj_qt(qt, tail=False):
        psy = [psA.tile([128, GF], F32, tag="mm", name=f"psy{qt}_{i}") for i in range(2)]
        if tail:
            # fold the bias in as a rank-1 matmul so the evacuation is a pure
            # copy that can run on ACT — keeps the drain off the busy DVE
            for oc in range(2):
                nc.tensor.matmul(
                    psy[oc][:], lhsT=ones128[:], rhs=pb_bf[:, ds(oc * GF, GF)],
                    start=True, stop=False, skip_group_check=True,
                )
        for f in range(FP):
            for oc in range(2):
                nc.tensor.matmul(
                    psy[oc][:],
                    lhsT=outT_sb[:, f, ts(qt, 128)],
                    rhs=wp_sb[:, f, ds(oc * GF, GF)],
                    start=(f == 0 and not tail),
                    stop=(f == FP - 1),
                    skip_group_check=True,
                )
        y_sb = yp.tile([128, C], F32, tag="y", name=f"y{qt}")
        for oc in range(2):
            if tail:
                nc.scalar.copy(y_sb[:, ds(oc * GF, GF)], psy[oc][:])
            else:
                nc.vector.tensor_add(
                    y_sb[:, ds(oc * GF, GF)], psy[oc][:], pb_sb[:, ds(oc * GF, GF)]
                )
            nc.sync.dma_start(
                out=out.ap()[ts(qt, 128), ds(oc * GF, GF)],
                in_=y_sb[:, ds(oc * GF, GF)],
            )

    from collections import deque

    # ---- upfront PE work, ordered by which xT chunk unblocks it (PE is
    # FIFO: anything emitted behind a stalled group head-of-line blocks)
    qk_group(wq_sb, 0, 0, qT_sb, True)
    qk_group(wk_sb, 0, 0, kT_sb, False)
    for nt in range(4):  # v tiles 0-3 need only xT chunk 0
        v_group(nt)
    for nb in range(1, NB):
        qk_group(wq_sb, 0, nb, qT_sb, True)
        qk_group(wk_sb, 0, nb, kT_sb, False)
    v_next = [4]  # next v tile to emit as priority fill

    # background PE fill: remaining q/k feature blocks (f=1 then f=2), one
    # PSUM-group per item; consumed during the exp-paced attention phase.
    fillq = deque()
    for f in range(1, FP):
        for nb in range(NB):
            fillq.append((wq_sb, f, nb, qT_sb, True))
        for nb in range(NB):
            fillq.append((wk_sb, f, nb, kT_sb, False))

    avq = deque()      # (block, kt) awaiting attn@v emission
    projq = deque()    # qt tiles awaiting projection
    norm_pending = {}  # nb -> count of hp norms done

    def drain_av_one():
        blk, kt = avq.popleft()
        # PE executes in emission order: v tile kt must be emitted first
        while v_next[0] <= kt:
            v_group(v_next[0])
            v_next[0] += 1
        emit_av(blk, kt)
        if kt == KT - 1:
            nb = blk.nb
            norm_pending[nb] = norm_pending.get(nb, 0) + 1
            if (blk.nb, blk.hp) == (NB - 1, FP - 1):
                emit_final_norm_proj(blk)
            else:
                emit_sums_copy(blk)
                emit_norm(blk)
                if norm_pending[nb] == FP:
                    projq.extend(range(4 * nb, 4 * nb + 4))

    def pump_fill():
        if v_next[0] < NT:
            v_group(v_next[0])
            v_next[0] += 1
        elif projq:
            emit_proj_qt(projq.popleft())
        elif fillq:
            qk_group(*fillq.popleft())

    # ---- attention block order: hp=0 across all nb first (only f=0 needed,
    # f=1/2 computed as fill work meanwhile), then hp=1,2 interleaved per nb
    # so each nb's projection unlocks early and spreads across the run.
    order = [(nb, 0) for nb in range(NB)]
    for nb in range(NB):
        order += [(nb, 1), (nb, 2)]
    for nb, hp in order:
        # scores below read feature tile f=hp: force any pending q/k fill
        # for f<=hp out now (normally long since drained via pump_fill)
        while any(item[1] <= hp for item in fillq):
            qk_group(*fillq.popleft())
        blk = Block(nb, hp)
        for kt in range(KT):
            emit_scores(blk, kt)
            avq.append((blk, kt))
            if len(avq) >= AVG + PAIR_LAG:
                for _ in range(AVG):
                    drain_av_one()
            pump_fill()
    while avq:
        drain_av_one()
    while projq:
        emit_proj_qt(projq.popleft())
    while fillq:
        qk_group(*fillq.popleft())


def build(krep=1):
    nc = bacc.Bacc("TRN2", target_bir_lowering=False, debug=False, num_devices=N_CORES)
    xT = nc.dram_tensor("xT", [C, N], BF16, kind="ExternalInput")
    wqT = nc.dram_tensor("wqT", [128, FP, CP, 128], BF16, kind="ExternalInput")
    wkT = nc.dram_tensor("wkT", [128, FP, CP, 128], BF16, kind="ExternalInput")
    wvT = nc.dram_tensor("wvT", [C, GF], BF16, kind="ExternalInput")
    wpT = nc.dram_tensor("wpT", [GF, C], BF16, kind="ExternalInput")
    qb = nc.dram_tensor("qb", [GF], F32, kind="ExternalInput")
    pb = nc.dram_tensor("pb", [C], F32, kind="ExternalInput")
    out = nc.dram_tensor("out", [N, C], F32, kind="ExternalOutput")
    aps = (xT, wqT, wkT, wvT, wpT, qb, pb, out)

    with tile.TileContext(nc) as tc:
        with (
            tc.tile_pool(name="const", bufs=1) as const,
            tc.tile_pool(name="qkv", bufs=1) as qkvp,
            tc.tile_pool(name="p", bufs=2) as pp,
            tc.tile_pool(name="norm", bufs=3) as normp,
            tc.tile_pool(name="y", bufs=3) as yp,
            tc.tile_pool(name="psA", bufs=2, space="PSUM") as psA,
            tc.tile_pool(name="psS", bufs=2, space="PSUM") as psS,
            tc.tile_pool(name="psO", bufs=2, space="PSUM") as psO,
            tc.tile_pool(name="dram", bufs=4, space="DRAM") as dramp,
        ):
            pools = (const, qkvp, pp, normp, yp, psA, psS, psO, dramp)
            for _ in range(krep):
                _body(nc, tc, pools, aps)
    nc.compile()
    return nc


def make_in_maps(x, qkv_weight, q_bias, v_bias, proj_weight, proj_bias):
    bf = ml_dtypes.bfloat16
    f32 = np.float32
    in_maps = []
    for c in range(N_CORES):
        b, g = c // 2, c % 2
        sl = slice(g * GF, (g + 1) * GF)
        def fmajor(wT):  # [C, GF] -> [128, FP, CP, 128] matching SBUF layout
            return np.ascontiguousarray(
                wT.reshape(CP, 128, FP, 128).transpose(1, 2, 0, 3)
            )

        wq = fmajor((qkv_weight[sl, :] * SCALE).T.astype(bf))
        wk = fmajor(qkv_weight[C + g * GF : C + (g + 1) * GF, :].T.astype(bf))
        wv = np.ascontiguousarray(qkv_weight[2 * C + g * GF : 2 * C + (g + 1) * GF, :].T).astype(bf)
        wp = np.ascontiguousarray(proj_weight[:, sl].T).astype(bf)
        qb_ = (q_bias[sl] * SCALE).astype(f32)
        vb_ = v_bias[sl].astype(np.float64)
        pb_ = (proj_weight[:, sl].astype(np.float64) @ vb_).astype(f32)
        if g == 0:
            pb_ = (pb_ + proj_bias).astype(f32)
        in_maps.append(
            dict(
                xT=np.ascontiguousarray(x[b].T).astype(bf),
                wqT=wq, wkT=wk, wvT=wv, wpT=wp,
                qb=np.ascontiguousarray(qb_), pb=np.ascontiguousarray(pb_),
            )
        )
    return in_maps


def gather(results):
    out = np.empty((B, N, C), np.float32)
    for b in range(B):
        out[b] = results[2 * b]["out"] + results[2 * b + 1]["out"]
    return out


_NC_CACHE = {}


def kernel(x, qkv_weight, q_bias, v_bias, proj_weight, proj_bias):
    if "nc" not in _NC_CACHE:
        _NC_CACHE["nc"] = build()
    nc = _NC_CACHE["nc"]
    in_maps = make_in_maps(x, qkv_weight, q_bias, v_bias, proj_weight, proj_bias)
    res = run_bass_kernel_spmd(nc, in_maps, core_ids=list(range(N_CORES)))
    return gather(res.results)


if __name__ == "__main__":
    rng = np.random.default_rng(0)
    x = rng.standard_normal((B, N, C), dtype=np.float32)
    qkv_weight = rng.standard_normal((3 * C, C), dtype=np.float32) * C**-0.5
    q_bias = rng.standard_normal(C, dtype=np.float32) * 0.02
    v_bias = rng.standard_normal(C, dtype=np.float32) * 0.02
    proj_weight = rng.standard_normal((C, C), dtype=np.float32) * C**-0.5
    proj_bias = rng.standard_normal(C, dtype=np.float32) * 0.02
    out = kernel(x, qkv_weight, q_bias, v_bias, proj_weight, proj_bias)
    print("out", out.shape, out.dtype, float(np.abs(out).mean()))

